# revision 17
# baseline (speedup 1.0000x reference)
"""NTM memory-addressing kernel (scatter_memory) for Trainium2, 8 NeuronCores.

Problem: B=2048, N=512, U=64 fp32.
  sim   = cosine_sim(memory, k)            (B, N)
  wc    = softmax(beta * sim)
  wg    = g*wc + (1-g)*w_pre
  ws    = circular 3-tap conv of wg with s
  w     = ws**gamma / sum + 1e-16
  r     = einsum('bn,bnu->bu', w, memory)
  new_m = memory * (1 - w e^T) + w a^T

Strategy: pure batch data-parallel across 8 cores (256 samples each). On-chip
work happens in a memory-TRANSPOSED layout (per sample M^T is [U=64, N=512]);
two samples pack the 128 partitions.  That makes the u-reductions (dot, norm)
PE matmuls with tiny block-diagonal stationary operands, lands the per-sample
stats directly in [sample, N] rows for the softmax/conv pipeline, and makes
the write-phase outer products PE matmuls that accumulate in PSUM on top of a
single fused DVE pass.  Host transposes memory/new_mem (free vs HW time).
"""

import numpy as np

B, N, U = 2048, 512, 64
NCORES = 8
SPC = B // NCORES  # samples per core


def _build_nc(spc, group):
    """Build the per-core Bass program. Identical program on all cores."""
    import concourse.bacc as bacc
    import concourse.mybir as mybir
    import concourse.tile as tile
    from contextlib import ExitStack

    fp32 = mybir.dt.float32
    AF = mybir.ActivationFunctionType
    OP = mybir.AluOpType
    AX = mybir.AxisListType

    ntiles = spc // 2
    gtiles = group // 2
    ngroups = spc // group
    assert ngroups * group == spc and gtiles * 2 == group

    nc = bacc.Bacc(
        "TRN2",
        target_bir_lowering=False,
        debug=False,
        enable_asserts=False,
        num_devices=NCORES,
    )

    mem_t = nc.dram_tensor("mem_t", [spc, U, N], fp32, kind="ExternalInput")
    wpre = nc.dram_tensor("wpre", [spc, N], fp32, kind="ExternalInput")
    krows = nc.dram_tensor("krows", [spc, U], fp32, kind="ExternalInput")
    s3 = nc.dram_tensor("s3", [spc, 3], fp32, kind="ExternalInput")
    kblk = nc.dram_tensor("kblk", [128, ngroups, gtiles, group], fp32,
                          kind="ExternalInput")
    oneb = nc.dram_tensor("oneb", [128, gtiles, group], fp32, kind="ExternalInput")
    eblk = nc.dram_tensor("eblk", [ntiles, 2, 128], fp32, kind="ExternalInput")
    ablk = nc.dram_tensor("ablk", [ntiles, 2, 128], fp32, kind="ExternalInput")
    ones2 = nc.dram_tensor("ones2", [2, 128], fp32, kind="ExternalInput")
    scal = nc.dram_tensor("scal", [128, 4], fp32, kind="ExternalInput")

    new_t = nc.dram_tensor("new_t", [spc, U, N], fp32, kind="ExternalOutput")
    w_out = nc.dram_tensor("w_out", [spc, N], fp32, kind="ExternalOutput")
    r_t = nc.dram_tensor("r_t", [128, ntiles], fp32, kind="ExternalOutput")

    with tile.TileContext(nc) as tc, ExitStack() as ctx:
        cpool = ctx.enter_context(tc.tile_pool(name="cpool", bufs=1))
        mpool = ctx.enter_context(tc.tile_pool(name="mpool", bufs=min(ntiles, gtiles + 12)))
        stpool = ctx.enter_context(tc.tile_pool(name="stpool", bufs=3))
        kpool = ctx.enter_context(tc.tile_pool(name="kpool", bufs=2))
        s2pool = ctx.enter_context(tc.tile_pool(name="s2pool", bufs=2))
        s3pool = ctx.enter_context(tc.tile_pool(name="s3pool", bufs=6))
        scrpool = ctx.enter_context(tc.tile_pool(name="scrpool", bufs=2))
        nspool = ctx.enter_context(tc.tile_pool(name="nspool", bufs=4))
        pstat = ctx.enter_context(tc.tile_pool(name="pstat", bufs=1, space="PSUM"))
        pnewa = ctx.enter_context(tc.tile_pool(name="pnewa", bufs=2, space="PSUM"))
        pnewb = ctx.enter_context(tc.tile_pool(name="pnewb", bufs=2, space="PSUM"))
        pwb = ctx.enter_context(tc.tile_pool(name="pwb", bufs=2, space="PSUM"))
        dpool = ctx.enter_context(tc.tile_pool(name="dpool", bufs=2, space="DRAM"))

        # constants
        oneb_sb = cpool.tile([128, gtiles * group], fp32, name="oneb_sb")
        nc.sync.dma_start(oneb_sb, oneb.ap())
        scalb = cpool.tile([128, 4], fp32, name="scalb")
        nc.sync.dma_start(scalb, scal.ap())
        ones2_sb = cpool.tile([2, 128], fp32, name="ones2_sb")
        nc.sync.dma_start(ones2_sb, ones2.ap())
        rall = cpool.tile([128, ntiles], fp32, name="rall")

        for grp in range(ngroups):
            base = grp * group
            # ---------------- stage 1: load M^T, dot & norm stats via PE ----
            kblk_sb = kpool.tile([128, gtiles * group], fp32, tag="kblk",
                                 name=f"kblk_{grp}")
            nc.sync.dma_start(kblk_sb, kblk.ap()[:, grp])
            dot_ps = pstat.tile([group, N], fp32, tag="dot", name=f"dot_{grp}")
            nrm_ps = pstat.tile([group, N], fp32, tag="nrm", name=f"nrm_{grp}")
            mts = []
            for tl in range(gtiles):
                t = grp * gtiles + tl
                mt = mpool.tile([128, N], fp32, tag="mt", name=f"mt_{t}")
                nc.sync.dma_start(
                    mt, mem_t.ap()[2 * t:2 * t + 2].rearrange("b u n -> (b u) n"))
                st = stpool.tile([128, N], fp32, tag="st", name=f"st_{t}")
                nc.scalar.activation(st, mt, AF.Square)
                nc.tensor.matmul(dot_ps, kblk_sb[:, tl * group:(tl + 1) * group],
                                 mt, start=(tl == 0), stop=(tl == gtiles - 1))
                nc.tensor.matmul(nrm_ps, oneb_sb[:, tl * group:(tl + 1) * group],
                                 st, start=(tl == 0), stop=(tl == gtiles - 1))
                mts.append(mt)

            # ---------------- stage 2: per-sample pipeline on [group, N] ----
            kg = s2pool.tile([group, U], fp32, tag="kg", name=f"kg_{grp}")
            nc.sync.dma_start(kg, krows.ap()[base:base + group])
            k2t = s2pool.tile([group, U], fp32, tag="k2t", name=f"k2t_{grp}")
            k2 = s2pool.tile([group, 1], fp32, tag="k2", name=f"k2_{grp}")
            nc.scalar.activation(k2t, kg, AF.Square, accum_out=k2)
            sg = s2pool.tile([group, 3], fp32, tag="sg", name=f"sg_{grp}")
            nc.sync.dma_start(sg, s3.ap()[base:base + group])
            wpg = s2pool.tile([group, N], fp32, tag="wpg", name=f"wpg_{grp}")
            nc.sync.dma_start(wpg, wpre.ap()[base:base + group])

            # q := 1/sqrt(norm2 * k2)  (rsqrt via exp(-0.5*ln(x)))
            q = s2pool.tile([group, N], fp32, tag="q", name=f"q_{grp}")
            nc.scalar.mul(q, nrm_ps, k2)
            nc.scalar.activation(q, q, AF.Ln)
            nc.scalar.activation(q, q, AF.Exp, scale=-0.5)
            # sim, then unnormalized content weights exp(beta*sim)
            simt = s2pool.tile([group, N], fp32, tag="simt", name=f"simt_{grp}")
            nc.vector.tensor_tensor(simt, dot_ps, q, op=OP.mult)
            nc.scalar.activation(simt, simt, AF.Exp, scale=scalb[0:group, 0:1])
            den = s2pool.tile([group, 1], fp32, tag="den", name=f"den_{grp}")
            nc.vector.tensor_reduce(den, simt, axis=AX.X, op=OP.add)
            rd = s2pool.tile([group, 1], fp32, tag="rd", name=f"rd_{grp}")
            nc.vector.reciprocal(rd, den)
            gd = s2pool.tile([group, 1], fp32, tag="gd", name=f"gd_{grp}")
            nc.vector.tensor_tensor(gd, rd, scalb[0:group, 1:2], op=OP.mult)
            # wg = wc*g + (1-g)*w_pre, written into padded buffer for the conv
            nc.scalar.mul(wpg, wpg, scalb[0:group, 2:3])
            wgp = s2pool.tile([group, N + 2], fp32, tag="wgp", name=f"wgp_{grp}")
            nc.vector.scalar_tensor_tensor(wgp[:, 1:N + 1], simt, gd, wpg,
                                           op0=OP.mult, op1=OP.add)
            nc.vector.tensor_copy(wgp[:, N + 1:N + 2], wgp[:, 1:2])
            nc.vector.tensor_copy(wgp[:, 0:1], wgp[:, N:N + 1])
            # circular 3-tap conv with per-sample taps
            ws = s2pool.tile([group, N], fp32, tag="ws", name=f"ws_{grp}")
            nc.vector.tensor_scalar(ws, wgp[:, 0:N], sg[:, 0:1], None, op0=OP.mult)
            nc.vector.scalar_tensor_tensor(ws, wgp[:, 1:N + 1], sg[:, 1:2], ws,
                                           op0=OP.mult, op1=OP.add)
            nc.vector.scalar_tensor_tensor(ws, wgp[:, 2:N + 2], sg[:, 2:3], ws,
                                           op0=OP.mult, op1=OP.add)
            # sharpen: ws**gamma = exp(gamma*ln(ws)), then normalize, +1e-16
            nc.scalar.activation(ws, ws, AF.Ln)
            nc.scalar.activation(ws, ws, AF.Exp, scale=scalb[0:group, 3:4])
            dn = s2pool.tile([group, 1], fp32, tag="dn", name=f"dn_{grp}")
            nc.vector.tensor_reduce(dn, ws, axis=AX.X, op=OP.add)
            rdn = s2pool.tile([group, 1], fp32, tag="rdn", name=f"rdn_{grp}")
            nc.vector.reciprocal(rdn, dn)
            wgrp = s2pool.tile([group, N], fp32, tag="wgrp", name=f"wgrp_{grp}")
            nc.vector.tensor_scalar(wgrp, ws, rdn, 1e-16, op0=OP.mult, op1=OP.add)
            nc.sync.dma_start(w_out.ap()[base:base + group], wgrp)
            # round-trip w through DRAM to get sample-pair rows on partitions 0:2
            wscr = dpool.tile([group, N], fp32, tag="wscr", name=f"wscr_{grp}")
            nc.sync.dma_start(wscr, wgrp)

            # ---------------- stage 3: write phase + read vector ------------
            for tl in range(gtiles):
                t = grp * gtiles + tl
                mt = mts[tl]
                w2t = s3pool.tile([2, N], fp32, tag="w2t", name=f"w2t_{t}")
                nc.sync.dma_start(w2t, wscr[2 * tl:2 * tl + 2, :])
                ebt = s3pool.tile([2, 128], fp32, tag="ebt", name=f"ebt_{t}")
                nc.sync.dma_start(ebt, eblk.ap()[t])
                abt = s3pool.tile([2, 128], fp32, tag="abt", name=f"abt_{t}")
                nc.sync.dma_start(abt, ablk.ap()[t])
                # new = M*(1 - e w) + a w, built in one PSUM bank:
                #   PE writes -(e x w) (sets has_written), DVE fuses (F+1)*M
                #   in place, PE accumulates a x w on top.
                fpa = pnewa.tile([128, N], fp32, tag="fpa", name=f"fpa_{t}")
                nc.tensor.matmul(fpa, ebt, w2t, start=True, stop=True)
                newp = pnewb.tile([128, N], fp32, tag="newp", name=f"newp_{t}")
                # second F' write only to set has_written on bank B; its
                # contents are overwritten by the fused DVE op below.
                nc.tensor.matmul(newp, ebt, w2t, start=True, stop=True)
                nc.vector.scalar_tensor_tensor(newp, fpa, 1.0, mt,
                                               op0=OP.add, op1=OP.mult)
                nc.tensor.matmul(newp, abt, w2t, start=False, stop=True,
                                 skip_group_check=True)
                # r = sum_n w[n] * M^T[:, n]  via fused multiply+reduce
                wb = pwb.tile([128, N], fp32, tag="wb", name=f"wb_{t}")
                nc.tensor.matmul(wb, ones2_sb, w2t, start=True, stop=True)
                scr = scrpool.tile([128, N], fp32, tag="scr", name=f"scr_{t}")
                nc.vector.scalar_tensor_tensor(
                    scr, mt, 1.0, wb, op0=OP.mult, op1=OP.mult,
                    accum_out=rall[:, t:t + 1])
                # evacuate PSUM -> SBUF -> DRAM
                nsb = nspool.tile([128, N], fp32, tag="nsb", name=f"nsb_{t}")
                nc.scalar.copy(nsb, newp)
                nc.sync.dma_start(
                    new_t.ap()[2 * t:2 * t + 2].rearrange("b u n -> (b u) n"), nsb)

        nc.sync.dma_start(r_t.ap(), rall)

    nc.compile()
    return nc


def _host_blocks(k_sh, e_sh, a_sh, spc, group):
    """Build the block-diagonal stationary operands on the host."""
    ntiles = spc // 2
    gtiles = group // 2
    ngroups = spc // group
    kk = k_sh.reshape(ngroups, gtiles, 2, U)
    kblk = np.zeros((128, ngroups, gtiles, group), np.float32)
    for tl in range(gtiles):
        kblk[0:64, :, tl, 2 * tl] = kk[:, tl, 0, :].T
        kblk[64:128, :, tl, 2 * tl + 1] = kk[:, tl, 1, :].T
    eblk = np.zeros((ntiles, 2, 128), np.float32)
    eblk[:, 0, 0:64] = -e_sh[0::2]
    eblk[:, 1, 64:128] = -e_sh[1::2]
    ablk = np.zeros((ntiles, 2, 128), np.float32)
    ablk[:, 0, 0:64] = a_sh[0::2]
    ablk[:, 1, 64:128] = a_sh[1::2]
    return kblk.reshape(128, ngroups, gtiles, group), eblk, ablk


def _oneb_const(group):
    gtiles = group // 2
    oneb = np.zeros((128, gtiles, group), np.float32)
    for tl in range(gtiles):
        oneb[0:64, tl, 2 * tl] = 1.0
        oneb[64:128, tl, 2 * tl + 1] = 1.0
    return oneb


def make_in_maps(memory, k, beta, g, s, gamma, w_pre, e, a, spc=SPC, group=64,
                 ncores=NCORES):
    memory = np.asarray(memory, np.float32)
    k = np.asarray(k, np.float32)
    s = np.asarray(s, np.float32)
    w_pre = np.asarray(w_pre, np.float32)
    e = np.asarray(e, np.float32)
    a = np.asarray(a, np.float32)
    bval = float(np.asarray(beta).reshape(-1)[0])
    gval = float(np.asarray(g).reshape(-1)[0])
    gaval = float(np.asarray(gamma).reshape(-1)[0])
    scal = np.broadcast_to(
        np.array([[bval, gval, 1.0 - gval, gaval]], np.float32), (128, 4)).copy()
    oneb = _oneb_const(group)
    ones2 = np.zeros((2, 128), np.float32)
    ones2[0, 0:64] = 1.0
    ones2[1, 64:128] = 1.0
    in_maps = []
    for c in range(ncores):
        sl = slice(c * spc, (c + 1) * spc)
        kblk, eblk, ablk = _host_blocks(k[sl], e[sl], a[sl], spc, group)
        in_maps.append(dict(
            mem_t=np.ascontiguousarray(memory[sl].transpose(0, 2, 1)),
            wpre=np.ascontiguousarray(w_pre[sl]),
            krows=np.ascontiguousarray(k[sl]),
            s3=np.ascontiguousarray(s[sl]),
            kblk=kblk, oneb=oneb, eblk=eblk, ablk=ablk, ones2=ones2, scal=scal,
        ))
    return in_maps


def assemble_outputs(results, spc=SPC, ncores=NCORES):
    ntiles = spc // 2
    Btot = spc * ncores
    w_full = np.empty((Btot, N), np.float32)
    r_full = np.empty((Btot, U), np.float32)
    new_full = np.empty((Btot, N, U), np.float32)
    for c, res in enumerate(results):
        sl = slice(c * spc, (c + 1) * spc)
        w_full[sl] = res["w_out"]
        rt = res["r_t"].reshape(2, 64, ntiles)
        r_full[sl] = rt.transpose(2, 0, 1).reshape(spc, U)
        new_full[sl] = res["new_t"].transpose(0, 2, 1)
    return w_full, r_full, new_full


_NC_CACHE = {}


def _get_nc(spc=SPC, group=64):
    key = (spc, group)
    if key not in _NC_CACHE:
        _NC_CACHE[key] = _build_nc(spc, group)
    return _NC_CACHE[key]


def kernel(memory, k, beta, g, s, gamma, w_pre, e, a):
    from concourse.bass_utils import run_bass_kernel_spmd

    nc = _get_nc()
    in_maps = make_in_maps(memory, k, beta, g, s, gamma, w_pre, e, a)
    out = run_bass_kernel_spmd(nc, in_maps, core_ids=list(range(NCORES)))
    return assemble_outputs(out.results)
